# revision 4
# baseline (speedup 1.0000x reference)
"""MHA kernel for Trainium2, 8 NeuronCores.

Problem: B=4, T=2048, D=1024, H=16, HD=64 fp32 multi-head attention
  qkv = x @ w_qkv ; attention per head ; out = y @ w_o

Sharding: core c handles batch b = c//2 and head-group g = c%2 (8 of the 16
heads). Each core computes its 8 heads' attention output projected through
the matching w_o row-slice, producing a partial [T, D] output; the host sums
the two partials per batch (row-parallel output projection).

Per-core pipeline (all matmuls on PE in float32r, 1 cycle/row):
  XT = x.T                (PE transpose via identity)
  QT/KT = (w_q|w_k).T xT  ([j, t] orientation, f16)
  V = x w_v               ([t, j] natural, f16, with interleaved ones cols)
  per head, per t-block:
    scoresT[s, t] = KT.T QT    (f16 in, f32 psum)
    expT = exp(0.125 * scoresT)  (ACT, f16 out)
    yu[65, t] = [V|1].T expT     (psum accumulate over s; row 64 = denom)
    y = yu[0:64] * broadcast(1/denom)   (DVE + PE broadcast)
  out = Y.T w_o            (accumulate 4 dy-chunks in psum)
"""
import sys

if "/opt/trn_rl_repo" not in sys.path:
    sys.path.insert(0, "/opt/trn_rl_repo")

import numpy as np

import concourse.bass as bass
import concourse.mybir as mybir
import concourse.tile as tile
from concourse import bacc
from concourse.bass_utils import run_bass_kernel_spmd
from concourse.masks import make_identity

T = 2048
D = 1024
NH = 8          # heads per core
HD = 64
KC = D // 128   # 8 contraction chunks
TT = T // 128   # 16 t/s tiles
F32 = mybir.dt.float32
F32R = mybir.dt.float32r
F16 = mybir.dt.float16

_CACHE = {}


def build_nc():
    nc = bacc.Bacc(
        "TRN2",
        target_bir_lowering=False,
        debug=False,
        enable_asserts=False,
        num_devices=8,
    )
    x_d = nc.dram_tensor("x", [T, D], F32R, kind="ExternalInput")
    wq_d = nc.dram_tensor("wq", [D, 512], F32R, kind="ExternalInput")
    wk_d = nc.dram_tensor("wk", [D, 512], F32R, kind="ExternalInput")
    wv_d = nc.dram_tensor("wv", [D, 512], F32R, kind="ExternalInput")
    wo_d = nc.dram_tensor("wo", [512, D], F16, kind="ExternalInput")
    out_d = nc.dram_tensor("out", [T, D], F32, kind="ExternalOutput")

    x_ap = x_d.ap()                                        # [2048, 1024]
    wq_ap = wq_d.ap().rearrange("(kc p) j -> p kc j", p=128)   # [128, 8, 512]
    wk_ap = wk_d.ap().rearrange("(kc p) j -> p kc j", p=128)
    wv_ap = wv_d.ap().rearrange("(kc p) j -> p kc j", p=128)
    wo_ap = wo_d.ap().rearrange("(c p) n -> p c n", p=128)     # [128, 4, 1024]

    with tile.TileContext(nc) as tc:
        with tc.sbuf_pool(name="persist", bufs=1) as pers:
            # ---- persistent sbuf tensors ----
            qkt = pers.tile([128, 8, T], F16)        # QT (jt 0-3) / KT (jt 4-7)
            vones = pers.tile([128, TT, NH * 65], F16)
            yt = pers.tile([128, 4, T], F16)         # normalized Y^T

            ones_f32 = pers.tile([1, HD], F32)
            nc.vector.memset(ones_f32, 1.0)
            ones_col = pers.tile([1, HD], F32R)
            nc.vector.tensor_copy(out=ones_col, in_=ones_f32)

            # ones columns of vones (col 64 of each head's 65-col group)
            vones_h = vones.rearrange("p s (h c) -> p s h c", c=65)
            nc.vector.memset(vones_h[:, :, :, 64:65], 1.0)

            # ---- phase 0 + 1: XT, then QKT / V ----
            with (
                tc.sbuf_pool(name="sb01", bufs=1) as sb01,
                tc.psum_pool(name="ps01", bufs=1) as ps01,
            ):
                ident_f32 = sb01.tile([128, 128], F32)
                make_identity(nc, ident_f32)
                ident = sb01.tile([128, 128], F32R)
                nc.vector.tensor_copy(out=ident, in_=ident_f32)

                xt = sb01.tile([128, KC, T], F32R)   # x.T  [d, t]
                wqk_sb = sb01.tile([128, KC, 1024], F32R)
                wv_sb = sb01.tile([128, KC, 512], F32R)
                nc.sync.dma_start(out=wqk_sb[:, :, 0:512], in_=wq_ap)
                nc.sync.dma_start(out=wqk_sb[:, :, 512:1024], in_=wk_ap)
                nc.sync.dma_start(out=wv_sb, in_=wv_ap)

                for tt in range(TT):
                    x_nat = sb01.tile([128, D], F32R, tag="xnat", bufs=2)
                    nc.sync.dma_start(
                        out=x_nat, in_=x_ap[tt * 128:(tt + 1) * 128, :]
                    )
                    for q in range(2):
                        xt_ps = ps01.tile([128, 512], F32R, tag="xt", bufs=2)
                        for r in range(4):
                            kc = 4 * q + r
                            nc.tensor.transpose(
                                xt_ps[:, r * 128:(r + 1) * 128],
                                x_nat[:, kc * 128:(kc + 1) * 128],
                                ident,
                            )
                        nc.vector.tensor_copy(
                            out=xt[:, 4 * q:4 * q + 4, tt * 128:(tt + 1) * 128],
                            in_=xt_ps.rearrange("p (r t) -> p r t", t=128),
                        )

                # QT / KT : [j, t] via lhsT = w chunk, rhs = XT chunk
                for jt in range(8):
                    for tb in range(4):
                        qk_ps = ps01.tile([128, 512], F32, tag="qk", bufs=3)
                        for kc in range(KC):
                            nc.tensor.matmul(
                                qk_ps,
                                wqk_sb[:, kc, jt * 128:(jt + 1) * 128],
                                xt[:, kc, tb * 512:(tb + 1) * 512],
                                start=(kc == 0),
                                stop=(kc == KC - 1),
                            )
                        nc.vector.tensor_copy(
                            out=qkt[:, jt, tb * 512:(tb + 1) * 512], in_=qk_ps
                        )

                # V natural [t, j] via lhsT = XT chunk, rhs = wv chunk
                vones_v = vones_h[:, :, :, 0:64]   # [128, TT, 8, 64]
                for tt in range(TT):
                    v_ps = ps01.tile([128, 512], F32, tag="v", bufs=3)
                    for kc in range(KC):
                        nc.tensor.matmul(
                            v_ps,
                            xt[:, kc, tt * 128:(tt + 1) * 128],
                            wv_sb[:, kc, :],
                            start=(kc == 0),
                            stop=(kc == KC - 1),
                        )
                    nc.vector.tensor_copy(
                        out=vones_v[:, tt],
                        in_=v_ps.rearrange("p (h c) -> p h c", c=64),
                    )

            # ---- phase 2: attention per head ----
            with (
                tc.sbuf_pool(name="sb2", bufs=1) as sb2,
                tc.psum_pool(name="ps2", bufs=1) as ps2,
            ):
                for h in range(NH):
                    pb = 64 * (h % 2)
                    qt_h = qkt[pb:pb + 64, h // 2, :]
                    kt_h = qkt[pb:pb + 64, 4 + h // 2, :]
                    for tb in range(2):          # t blocks of 1024
                        t0 = tb * 1024
                        exp_tiles = []
                        for i in range(TT):
                            sc_ps = ps2.tile([128, 1024], F32, tag="sc", bufs=2)
                            for u in range(2):
                                nc.tensor.matmul(
                                    sc_ps[:, u * 512:(u + 1) * 512],
                                    kt_h[:, i * 128:(i + 1) * 128],
                                    qt_h[:, t0 + u * 512:t0 + (u + 1) * 512],
                                    start=True,
                                    stop=True,
                                )
                            e_sb = sb2.tile([128, 1024], F16, tag="exp", bufs=20)
                            nc.scalar.activation(
                                e_sb, sc_ps,
                                mybir.ActivationFunctionType.Exp,
                                scale=0.125,
                            )
                            exp_tiles.append(e_sb)

                        yu_ps = ps2.tile([65, 1024], F32, tag="yu", bufs=1)
                        for u in range(2):
                            for i in range(TT):
                                nc.tensor.matmul(
                                    yu_ps[:, u * 512:(u + 1) * 512],
                                    vones_h[:, i, h, :],
                                    exp_tiles[i][:, u * 512:(u + 1) * 512],
                                    start=(i == 0),
                                    stop=(i == TT - 1),
                                )

                        rec = sb2.tile([1, 1024], F32R, tag="rec", bufs=2)
                        with nc.allow_low_precision(reason="f32r recip"):
                            nc.vector.reciprocal(rec, yu_ps[64:65, :])
                        bc_ps = ps2.tile([64, 1024], F32, tag="bc", bufs=1)
                        for u in range(2):
                            nc.tensor.matmul(
                                bc_ps[:, u * 512:(u + 1) * 512],
                                ones_col,
                                rec[:, u * 512:(u + 1) * 512],
                                start=True,
                                stop=True,
                            )
                        rec_bc = sb2.tile([64, 1024], F32, tag="recbc", bufs=2)
                        nc.vector.tensor_copy(out=rec_bc, in_=bc_ps)
                        with nc.allow_low_precision(reason="f32r y"):
                            nc.vector.tensor_mul(
                                out=yt[pb:pb + 64, h // 2, t0:t0 + 1024],
                                in0=yu_ps[0:64, :],
                                in1=rec_bc,
                            )

            # ---- phase 3: output projection ----
            with (
                tc.sbuf_pool(name="sb3", bufs=1) as sb3,
                tc.psum_pool(name="ps3", bufs=1) as ps3,
            ):
                wo_sb = sb3.tile([128, 4, D], F16)
                nc.sync.dma_start(out=wo_sb, in_=wo_ap)
                for tt in range(TT):
                    o_ps = ps3.tile([128, D], F32, tag="ops", bufs=3)
                    for c4 in range(4):
                        for u in range(2):
                            nc.tensor.matmul(
                                o_ps[:, u * 512:(u + 1) * 512],
                                yt[:, c4, tt * 128:(tt + 1) * 128],
                                wo_sb[:, c4, u * 512:(u + 1) * 512],
                                start=(c4 == 0),
                                stop=(c4 == 3),
                            )
                    o_sb = sb3.tile([128, D], F32, tag="osb", bufs=3)
                    nc.vector.tensor_copy(out=o_sb, in_=o_ps)
                    nc.sync.dma_start(
                        out=out_d.ap()[tt * 128:(tt + 1) * 128, :], in_=o_sb
                    )

    nc.compile()
    return nc


def make_in_maps(x, w_qkv, w_o):
    in_maps = []
    for c in range(8):
        b, g = c // 2, c % 2
        in_maps.append({
            "x": np.ascontiguousarray(x[b], dtype=np.float32),
            "wq": np.ascontiguousarray(
                w_qkv[:, 512 * g:512 * (g + 1)], dtype=np.float32),
            "wk": np.ascontiguousarray(
                w_qkv[:, 1024 + 512 * g:1024 + 512 * (g + 1)], dtype=np.float32),
            "wv": np.ascontiguousarray(
                w_qkv[:, 2048 + 512 * g:2048 + 512 * (g + 1)], dtype=np.float32),
            "wo": np.ascontiguousarray(
                w_o[512 * g:512 * (g + 1), :], dtype=np.float16),
        })
    return in_maps


def kernel(x, w_qkv, w_o, _trace=False, _trace_kwargs=None):
    x = np.asarray(x)
    w_qkv = np.asarray(w_qkv)
    w_o = np.asarray(w_o)
    if "nc" not in _CACHE:
        _CACHE["nc"] = build_nc()
    nc = _CACHE["nc"]
    in_maps = make_in_maps(x, w_qkv, w_o)
    res = run_bass_kernel_spmd(
        nc, in_maps, core_ids=list(range(8)),
        trace=_trace, **(_trace_kwargs or {}),
    )
    out = np.empty((4, T, D), np.float32)
    for b in range(4):
        out[b] = res.results[2 * b]["out"] + res.results[2 * b + 1]["out"]
    if _trace:
        _CACHE["last_res"] = res
    return out


# revision 5
# speedup vs baseline: 2.4753x; 2.4753x over previous
"""MHA kernel for Trainium2, 8 NeuronCores.

Problem: B=4, T=2048, D=1024, H=16, HD=64 fp32 multi-head attention
  qkv = x @ w_qkv ; attention per head ; out = y @ w_o

Sharding: core c handles batch b = c//2 and head-group g = c%2 (8 of the 16
heads). Each core computes its 8 heads' attention output projected through
the matching w_o row-slice, producing a partial [T, D] output; the host sums
the two partials per batch (row-parallel output projection).

Per-core pipeline (all matmuls on PE in float32r, 1 cycle/row):
  XT = x.T                (PE transpose via identity)
  QT/KT = (w_q|w_k).T xT  ([j, t] orientation, f16)
  V = x w_v               ([t, j] natural, f16, with interleaved ones cols)
  per head, per t-block:
    scoresT[s, t] = KT.T QT    (f16 in, f32 psum)
    expT = exp(0.125 * scoresT)  (ACT, f16 out)
    yu[65, t] = [V|1].T expT     (psum accumulate over s; row 64 = denom)
    y = yu[0:64] * broadcast(1/denom)   (DVE + PE broadcast)
  out = Y.T w_o            (accumulate 4 dy-chunks in psum)
"""
import sys

if "/opt/trn_rl_repo" not in sys.path:
    sys.path.insert(0, "/opt/trn_rl_repo")

import numpy as np

import concourse.bass as bass
import concourse.mybir as mybir
import concourse.tile as tile
from concourse import bacc
from concourse.bass_utils import run_bass_kernel_spmd
from concourse.masks import make_identity

T = 2048
D = 1024
NH = 8          # heads per core
HD = 64
KC = D // 128   # 8 contraction chunks
TT = T // 128   # 16 t/s tiles
F32 = mybir.dt.float32
F32R = mybir.dt.float32r
F16 = mybir.dt.float16

_CACHE = {}


def build_nc():
    nc = bacc.Bacc(
        "TRN2",
        target_bir_lowering=False,
        debug=False,
        enable_asserts=False,
        num_devices=8,
    )
    x_d = nc.dram_tensor("x", [T, D], F32R, kind="ExternalInput")
    wq_d = nc.dram_tensor("wq", [D, 512], F32R, kind="ExternalInput")
    wk_d = nc.dram_tensor("wk", [D, 512], F32R, kind="ExternalInput")
    wv_d = nc.dram_tensor("wv", [D, 512], F32R, kind="ExternalInput")
    wo_d = nc.dram_tensor("wo", [512, D], F16, kind="ExternalInput")
    out_d = nc.dram_tensor("out", [T, D], F32, kind="ExternalOutput")

    x_ap = x_d.ap()                                        # [2048, 1024]
    wq_ap = wq_d.ap().rearrange("(kc p) j -> p kc j", p=128)   # [128, 8, 512]
    wk_ap = wk_d.ap().rearrange("(kc p) j -> p kc j", p=128)
    wv_ap = wv_d.ap().rearrange("(kc p) j -> p kc j", p=128)
    wo_ap = wo_d.ap().rearrange("(c p) n -> p c n", p=128)     # [128, 4, 1024]

    with tile.TileContext(nc) as tc:
        with tc.sbuf_pool(name="persist", bufs=1) as pers:
            # ---- persistent sbuf tensors ----
            qkt = pers.tile([128, 8, T], F16)        # QT (jt 0-3) / KT (jt 4-7)
            vones = pers.tile([128, TT, NH * 65], F16)
            yt = pers.tile([128, 4, T], F16)         # normalized Y^T

            ones_f32 = pers.tile([1, HD], F32)
            nc.vector.memset(ones_f32, 1.0)
            ones_col = pers.tile([1, HD], F32R)
            nc.vector.tensor_copy(out=ones_col, in_=ones_f32)

            # ones columns of vones (col 64 of each head's 65-col group)
            vones_h = vones.rearrange("p s (h c) -> p s h c", c=65)
            nc.vector.memset(vones_h[:, :, :, 64:65], 1.0)

            # ---- phase 0 + 1: XT, then QKT / V ----
            with (
                tc.sbuf_pool(name="sb01", bufs=1) as sb01,
                tc.psum_pool(name="ps01", bufs=1) as ps01,
            ):
                ident_f32 = sb01.tile([128, 128], F32)
                make_identity(nc, ident_f32)
                ident = sb01.tile([128, 128], F32R)
                nc.vector.tensor_copy(out=ident, in_=ident_f32)

                xt = sb01.tile([128, KC, T], F32R)   # x.T  [d, t]
                wqk_sb = sb01.tile([128, KC, 1024], F32R)
                wv_sb = sb01.tile([128, KC, 512], F32R)
                nc.sync.dma_start(out=wqk_sb[:, :, 0:512], in_=wq_ap)
                nc.sync.dma_start(out=wqk_sb[:, :, 512:1024], in_=wk_ap)
                nc.sync.dma_start(out=wv_sb, in_=wv_ap)

                for tt in range(TT):
                    x_nat = sb01.tile([128, D], F32R, tag="xnat", bufs=2)
                    nc.sync.dma_start(
                        out=x_nat, in_=x_ap[tt * 128:(tt + 1) * 128, :]
                    )
                    for q in range(2):
                        xt_ps = ps01.tile([128, 512], F32R, tag="xt", bufs=2)
                        for r in range(4):
                            kc = 4 * q + r
                            nc.tensor.transpose(
                                xt_ps[:, r * 128:(r + 1) * 128],
                                x_nat[:, kc * 128:(kc + 1) * 128],
                                ident,
                            )
                        nc.vector.tensor_copy(
                            out=xt[:, 4 * q:4 * q + 4, tt * 128:(tt + 1) * 128],
                            in_=xt_ps.rearrange("p (r t) -> p r t", t=128),
                        )

                # QT / KT : [j, t] via lhsT = w chunk, rhs = XT chunk
                for jt in range(8):
                    for tb in range(4):
                        qk_ps = ps01.tile([128, 512], F32, tag="qk", bufs=3)
                        for kc in range(KC):
                            nc.tensor.matmul(
                                qk_ps,
                                wqk_sb[:, kc, jt * 128:(jt + 1) * 128],
                                xt[:, kc, tb * 512:(tb + 1) * 512],
                                start=(kc == 0),
                                stop=(kc == KC - 1),
                            )
                        nc.vector.tensor_copy(
                            out=qkt[:, jt, tb * 512:(tb + 1) * 512], in_=qk_ps
                        )

                # V natural [t, j] via lhsT = XT chunk, rhs = wv chunk
                vones_v = vones_h[:, :, :, 0:64]   # [128, TT, 8, 64]
                for tt in range(TT):
                    v_ps = ps01.tile([128, 512], F32, tag="v", bufs=3)
                    for kc in range(KC):
                        nc.tensor.matmul(
                            v_ps,
                            xt[:, kc, tt * 128:(tt + 1) * 128],
                            wv_sb[:, kc, :],
                            start=(kc == 0),
                            stop=(kc == KC - 1),
                        )
                    nc.vector.tensor_copy(
                        out=vones_v[:, tt],
                        in_=v_ps.rearrange("p (h c) -> p h c", c=64),
                    )

            # ---- phase 2: attention per head ----
            with (
                tc.sbuf_pool(name="sb2", bufs=1) as sb2,
                tc.psum_pool(name="ps2", bufs=1) as ps2,
            ):
                for h in range(NH):
                    pb = 64 * (h % 2)
                    qt_h = qkt[pb:pb + 64, h // 2, :]
                    kt_h = qkt[pb:pb + 64, 4 + h // 2, :]
                    for tb in range(2):          # t blocks of 1024
                        t0 = tb * 1024
                        yu_ps = ps2.tile([65, 1024], F32, tag="yu", bufs=1)

                        def yu_mm(j):
                            for u in range(2):
                                nc.tensor.matmul(
                                    yu_ps[:, u * 512:(u + 1) * 512],
                                    vones_h[:, j, h, :],
                                    exp_tiles[j][:, u * 512:(u + 1) * 512],
                                    start=(j == 0),
                                    stop=(j == TT - 1),
                                    skip_group_check=True,
                                )

                        exp_tiles = []
                        for i in range(TT):
                            sc_ps = ps2.tile([128, 1024], F32, tag="sc", bufs=2)
                            for u in range(2):
                                nc.tensor.matmul(
                                    sc_ps[:, u * 512:(u + 1) * 512],
                                    kt_h[:, i * 128:(i + 1) * 128],
                                    qt_h[:, t0 + u * 512:t0 + (u + 1) * 512],
                                    start=True,
                                    stop=True,
                                )
                            e_sb = sb2.tile([128, 1024], F16, tag="exp", bufs=20)
                            nc.scalar.activation(
                                e_sb, sc_ps,
                                mybir.ActivationFunctionType.Exp,
                                scale=0.125,
                            )
                            exp_tiles.append(e_sb)
                            # interleave att@v accumulation one s-chunk behind
                            # the scores stream so PE never idles waiting on
                            # ACT (keeps HAM at full clock)
                            if i >= 1:
                                yu_mm(i - 1)
                        yu_mm(TT - 1)

                        rec = sb2.tile([1, 1024], F32R, tag="rec", bufs=2)
                        with nc.allow_low_precision(reason="f32r recip"):
                            nc.vector.reciprocal(rec, yu_ps[64:65, :])
                        bc_ps = ps2.tile([64, 1024], F32, tag="bc", bufs=1)
                        for u in range(2):
                            nc.tensor.matmul(
                                bc_ps[:, u * 512:(u + 1) * 512],
                                ones_col,
                                rec[:, u * 512:(u + 1) * 512],
                                start=True,
                                stop=True,
                            )
                        rec_bc = sb2.tile([64, 1024], F32, tag="recbc", bufs=2)
                        nc.vector.tensor_copy(out=rec_bc, in_=bc_ps)
                        with nc.allow_low_precision(reason="f32r y"):
                            nc.vector.tensor_mul(
                                out=yt[pb:pb + 64, h // 2, t0:t0 + 1024],
                                in0=yu_ps[0:64, :],
                                in1=rec_bc,
                            )

            # ---- phase 3: output projection ----
            with (
                tc.sbuf_pool(name="sb3", bufs=1) as sb3,
                tc.psum_pool(name="ps3", bufs=1) as ps3,
            ):
                wo_sb = sb3.tile([128, 4, D], F16)
                nc.sync.dma_start(out=wo_sb, in_=wo_ap)
                for tt in range(TT):
                    o_ps = ps3.tile([128, D], F32, tag="ops", bufs=3)
                    for c4 in range(4):
                        for u in range(2):
                            nc.tensor.matmul(
                                o_ps[:, u * 512:(u + 1) * 512],
                                yt[:, c4, tt * 128:(tt + 1) * 128],
                                wo_sb[:, c4, u * 512:(u + 1) * 512],
                                start=(c4 == 0),
                                stop=(c4 == 3),
                            )
                    o_sb = sb3.tile([128, D], F32, tag="osb", bufs=3)
                    nc.vector.tensor_copy(out=o_sb, in_=o_ps)
                    nc.sync.dma_start(
                        out=out_d.ap()[tt * 128:(tt + 1) * 128, :], in_=o_sb
                    )

    nc.compile()
    return nc


def make_in_maps(x, w_qkv, w_o):
    in_maps = []
    for c in range(8):
        b, g = c // 2, c % 2
        in_maps.append({
            "x": np.ascontiguousarray(x[b], dtype=np.float32),
            "wq": np.ascontiguousarray(
                w_qkv[:, 512 * g:512 * (g + 1)], dtype=np.float32),
            "wk": np.ascontiguousarray(
                w_qkv[:, 1024 + 512 * g:1024 + 512 * (g + 1)], dtype=np.float32),
            "wv": np.ascontiguousarray(
                w_qkv[:, 2048 + 512 * g:2048 + 512 * (g + 1)], dtype=np.float32),
            "wo": np.ascontiguousarray(
                w_o[512 * g:512 * (g + 1), :], dtype=np.float16),
        })
    return in_maps


def kernel(x, w_qkv, w_o, _trace=False, _trace_kwargs=None):
    x = np.asarray(x)
    w_qkv = np.asarray(w_qkv)
    w_o = np.asarray(w_o)
    if "nc" not in _CACHE:
        _CACHE["nc"] = build_nc()
    nc = _CACHE["nc"]
    in_maps = make_in_maps(x, w_qkv, w_o)
    res = run_bass_kernel_spmd(
        nc, in_maps, core_ids=list(range(8)),
        trace=_trace, **(_trace_kwargs or {}),
    )
    out = np.empty((4, T, D), np.float32)
    for b in range(4):
        out[b] = res.results[2 * b]["out"] + res.results[2 * b + 1]["out"]
    if _trace:
        _CACHE["last_res"] = res
    return out


# revision 7
# speedup vs baseline: 14160.5650x; 5720.7866x over previous
"""MHA kernel for Trainium2, 8 NeuronCores.

Problem: B=4, T=2048, D=1024, H=16, HD=64 fp32 multi-head attention
  qkv = x @ w_qkv ; attention per head ; out = y @ w_o

Sharding: core c handles batch b = c//2 and head-group g = c%2 (8 of the 16
heads). Each core computes its 8 heads' attention output projected through
the matching w_o row-slice, producing a partial [T, D] output; the host sums
the two partials per batch (row-parallel output projection).

Per-core pipeline (all matmuls on PE in float32r, 1 cycle/row):
  XT = x.T                (PE transpose via identity)
  QT/KT = (w_q|w_k).T xT  ([j, t] orientation, f16)
  V = x w_v               ([t, j] natural, f16, with interleaved ones cols)
  per head, per t-block:
    scoresT[s, t] = KT.T QT    (f16 in, f32 psum)
    expT = exp(0.125 * scoresT)  (ACT, f16 out)
    yu[65, t] = [V|1].T expT     (psum accumulate over s; row 64 = denom)
    y = yu[0:64] * broadcast(1/denom)   (DVE + PE broadcast)
  out = Y.T w_o            (accumulate 4 dy-chunks in psum)
"""
import sys

if "/opt/trn_rl_repo" not in sys.path:
    sys.path.insert(0, "/opt/trn_rl_repo")

import numpy as np

import concourse.bass as bass
import concourse.mybir as mybir
import concourse.tile as tile
from concourse import bacc
from concourse.bass_utils import run_bass_kernel_spmd
from concourse.masks import make_identity

T = 2048
D = 1024
NH = 8          # heads per core
HD = 64
KC = D // 128   # 8 contraction chunks
TT = T // 128   # 16 t/s tiles
F32 = mybir.dt.float32
F32R = mybir.dt.float32r
F16 = mybir.dt.float16

_CACHE = {}


def build_nc():
    nc = bacc.Bacc(
        "TRN2",
        target_bir_lowering=False,
        debug=False,
        enable_asserts=False,
        num_devices=8,
    )
    x_d = nc.dram_tensor("x", [T, D], F32R, kind="ExternalInput")
    wq_d = nc.dram_tensor("wq", [D, 512], F32R, kind="ExternalInput")
    wk_d = nc.dram_tensor("wk", [D, 512], F32R, kind="ExternalInput")
    wv_d = nc.dram_tensor("wv", [D, 512], F32R, kind="ExternalInput")
    wo_d = nc.dram_tensor("wo", [512, D], F16, kind="ExternalInput")
    out_d = nc.dram_tensor("out", [T, D], F32, kind="ExternalOutput")

    x_ap = x_d.ap()                                        # [2048, 1024]
    wq_ap = wq_d.ap().rearrange("(kc p) j -> p kc j", p=128)   # [128, 8, 512]
    wk_ap = wk_d.ap().rearrange("(kc p) j -> p kc j", p=128)
    wv_ap = wv_d.ap().rearrange("(kc p) j -> p kc j", p=128)
    wo_ap = wo_d.ap().rearrange("(c p) n -> p c n", p=128)     # [128, 4, 1024]

    with tile.TileContext(nc) as tc:
        with tc.sbuf_pool(name="persist", bufs=1) as pers:
            # ---- persistent sbuf tensors ----
            qkt = pers.tile([128, 8, T], F16)        # QT (jt 0-3) / KT (jt 4-7)
            vones = pers.tile([128, TT, NH * 65], F16)
            yt = pers.tile([128, 4, T], F16)         # normalized Y^T

            ones_f32 = pers.tile([1, HD], F32)
            nc.vector.memset(ones_f32, 1.0)
            ones_col = pers.tile([1, HD], F32R)
            nc.vector.tensor_copy(out=ones_col, in_=ones_f32)

            # ones columns of vones (col 64 of each head's 65-col group)
            vones_h = vones.rearrange("p s (h c) -> p s h c", c=65)
            nc.vector.memset(vones_h[:, :, :, 64:65], 1.0)

            # ---- phase 0 + 1: XT, then QKT / V ----
            with (
                tc.sbuf_pool(name="sb01", bufs=1) as sb01,
                tc.psum_pool(name="ps01", bufs=1) as ps01,
            ):
                ident_f32 = sb01.tile([128, 128], F32)
                make_identity(nc, ident_f32)
                ident = sb01.tile([128, 128], F32R)
                nc.vector.tensor_copy(out=ident, in_=ident_f32)

                xt = sb01.tile([128, KC, T], F32R)   # x.T  [d, t]
                wqk_sb = sb01.tile([128, KC, 1024], F32R)
                wv_sb = sb01.tile([128, KC, 512], F32R)
                nc.sync.dma_start(out=wqk_sb[:, :, 0:512], in_=wq_ap)
                nc.sync.dma_start(out=wqk_sb[:, :, 512:1024], in_=wk_ap)
                nc.sync.dma_start(out=wv_sb, in_=wv_ap)

                for tt in range(TT):
                    x_nat = sb01.tile([128, D], F32R, tag="xnat", bufs=2)
                    nc.sync.dma_start(
                        out=x_nat, in_=x_ap[tt * 128:(tt + 1) * 128, :]
                    )
                    for q in range(2):
                        xt_ps = ps01.tile([128, 512], F32R, tag="xt", bufs=2)
                        for r in range(4):
                            kc = 4 * q + r
                            nc.tensor.transpose(
                                xt_ps[:, r * 128:(r + 1) * 128],
                                x_nat[:, kc * 128:(kc + 1) * 128],
                                ident,
                            )
                        nc.vector.tensor_copy(
                            out=xt[:, 4 * q:4 * q + 4, tt * 128:(tt + 1) * 128],
                            in_=xt_ps.rearrange("p (r t) -> p r t", t=128),
                        )

                # QT / KT : [j, t] via lhsT = w chunk, rhs = XT chunk
                for jt in range(8):
                    for tb in range(4):
                        qk_ps = ps01.tile([128, 512], F32, tag="qk", bufs=3)
                        for kc in range(KC):
                            nc.tensor.matmul(
                                qk_ps,
                                wqk_sb[:, kc, jt * 128:(jt + 1) * 128],
                                xt[:, kc, tb * 512:(tb + 1) * 512],
                                start=(kc == 0),
                                stop=(kc == KC - 1),
                            )
                        nc.vector.tensor_copy(
                            out=qkt[:, jt, tb * 512:(tb + 1) * 512], in_=qk_ps
                        )

                # V natural [t, j] via lhsT = XT chunk, rhs = wv chunk
                vones_v = vones_h[:, :, :, 0:64]   # [128, TT, 8, 64]
                for tt in range(TT):
                    v_ps = ps01.tile([128, 512], F32, tag="v", bufs=3)
                    for kc in range(KC):
                        nc.tensor.matmul(
                            v_ps,
                            xt[:, kc, tt * 128:(tt + 1) * 128],
                            wv_sb[:, kc, :],
                            start=(kc == 0),
                            stop=(kc == KC - 1),
                        )
                    nc.vector.tensor_copy(
                        out=vones_v[:, tt],
                        in_=v_ps.rearrange("p (h c) -> p h c", c=64),
                    )

            # ---- phase 2: attention per head ----
            with (
                tc.sbuf_pool(name="sb2", bufs=1) as sb2,
                tc.psum_pool(name="ps2", bufs=1) as ps2,
            ):
                for h in range(NH):
                    pb = 64 * (h % 2)
                    qt_h = qkt[pb:pb + 64, h // 2, :]
                    kt_h = qkt[pb:pb + 64, 4 + h // 2, :]
                    for tb in range(2):          # t blocks of 1024
                        t0 = tb * 1024
                        yu_ps = ps2.tile([65, 1024], F32, tag="yu", bufs=2)

                        def yu_mm(j):
                            for u in range(2):
                                nc.tensor.matmul(
                                    yu_ps[:, u * 512:(u + 1) * 512],
                                    vones_h[:, j, h, :],
                                    exp_tiles[j][:, u * 512:(u + 1) * 512],
                                    start=(j == 0),
                                    stop=(j == TT - 1),
                                    skip_group_check=True,
                                )

                        exp_tiles = []
                        for i in range(TT):
                            sc_ps = ps2.tile([128, 1024], F32, tag="sc", bufs=2)
                            for u in range(2):
                                nc.tensor.matmul(
                                    sc_ps[:, u * 512:(u + 1) * 512],
                                    kt_h[:, i * 128:(i + 1) * 128],
                                    qt_h[:, t0 + u * 512:t0 + (u + 1) * 512],
                                    start=True,
                                    stop=True,
                                )
                            e_sb = sb2.tile([128, 1024], F16, tag="exp", bufs=20)
                            nc.scalar.activation(
                                e_sb, sc_ps,
                                mybir.ActivationFunctionType.Exp,
                                scale=0.125,
                            )
                            exp_tiles.append(e_sb)
                            # interleave att@v accumulation one s-chunk behind
                            # the scores stream so PE never idles waiting on
                            # ACT (keeps HAM at full clock)
                            if i >= 1:
                                yu_mm(i - 1)
                        yu_mm(TT - 1)

                        rec = sb2.tile([1, 1024], F32R, tag="rec", bufs=2)
                        with nc.allow_low_precision(reason="f32r recip"):
                            nc.vector.reciprocal(rec, yu_ps[64:65, :])
                        bc_full = ps2.tile([65, 1024], F32, tag="yu", bufs=2)
                        bc_ps = bc_full[0:64, :]
                        for u in range(2):
                            nc.tensor.matmul(
                                bc_ps[:, u * 512:(u + 1) * 512],
                                ones_col,
                                rec[:, u * 512:(u + 1) * 512],
                                start=True,
                                stop=True,
                            )
                        rec_bc = sb2.tile([64, 1024], F32, tag="recbc", bufs=2)
                        nc.vector.tensor_copy(out=rec_bc, in_=bc_ps)
                        with nc.allow_low_precision(reason="f32r y"):
                            nc.vector.tensor_mul(
                                out=yt[pb:pb + 64, h // 2, t0:t0 + 1024],
                                in0=yu_ps[0:64, :],
                                in1=rec_bc,
                            )

            # ---- phase 3: output projection ----
            with (
                tc.sbuf_pool(name="sb3", bufs=1) as sb3,
                tc.psum_pool(name="ps3", bufs=1) as ps3,
            ):
                wo_sb = sb3.tile([128, 4, D], F16)
                nc.sync.dma_start(out=wo_sb, in_=wo_ap)
                for tt in range(TT):
                    o_ps = ps3.tile([128, D], F32, tag="ops", bufs=3)
                    for c4 in range(4):
                        for u in range(2):
                            nc.tensor.matmul(
                                o_ps[:, u * 512:(u + 1) * 512],
                                yt[:, c4, tt * 128:(tt + 1) * 128],
                                wo_sb[:, c4, u * 512:(u + 1) * 512],
                                start=(c4 == 0),
                                stop=(c4 == 3),
                            )
                    o_sb = sb3.tile([128, D], F32, tag="osb", bufs=3)
                    nc.vector.tensor_copy(out=o_sb, in_=o_ps)
                    nc.sync.dma_start(
                        out=out_d.ap()[tt * 128:(tt + 1) * 128, :], in_=o_sb
                    )

    nc.compile()
    return nc


def make_in_maps(x, w_qkv, w_o):
    in_maps = []
    for c in range(8):
        b, g = c // 2, c % 2
        in_maps.append({
            "x": np.ascontiguousarray(x[b], dtype=np.float32),
            "wq": np.ascontiguousarray(
                w_qkv[:, 512 * g:512 * (g + 1)], dtype=np.float32),
            "wk": np.ascontiguousarray(
                w_qkv[:, 1024 + 512 * g:1024 + 512 * (g + 1)], dtype=np.float32),
            "wv": np.ascontiguousarray(
                w_qkv[:, 2048 + 512 * g:2048 + 512 * (g + 1)], dtype=np.float32),
            "wo": np.ascontiguousarray(
                w_o[512 * g:512 * (g + 1), :], dtype=np.float16),
        })
    return in_maps


def kernel(x, w_qkv, w_o, _trace=False, _trace_kwargs=None):
    x = np.asarray(x)
    w_qkv = np.asarray(w_qkv)
    w_o = np.asarray(w_o)
    if "nc" not in _CACHE:
        _CACHE["nc"] = build_nc()
    nc = _CACHE["nc"]
    in_maps = make_in_maps(x, w_qkv, w_o)
    res = run_bass_kernel_spmd(
        nc, in_maps, core_ids=list(range(8)),
        trace=_trace, **(_trace_kwargs or {}),
    )
    out = np.empty((4, T, D), np.float32)
    for b in range(4):
        out[b] = res.results[2 * b]["out"] + res.results[2 * b + 1]["out"]
    if _trace:
        _CACHE["last_res"] = res
    return out
